# revision 1
# baseline (speedup 1.0000x reference)
"""TRN2 Bass kernel for nn_HCSMoEQwen3MoeSparseMoeBlock (8-core expert-parallel).

Sharding: core g owns group g's dominant expert and processes ALL tokens;
router replicated (each core computes only its group's combined weight
w_g[t]); host sums the 8 partial outputs w_g[t] * y_g[t, :].

Single software-pipelined loop over 128-token chunks; float32r matmuls
(full PE rate, ~2e-4 rel err); router logits in exact fp32 (separate
F32-typed tiles — the PE precision mode follows the backing tensor dtype):
  router: logitsT = gwT.T-stationary @ x-chunk (fp32) -> PE transpose
  M1 b-major: h_b = xT_c.T @ gu_b, 16 same-bank MMs per 512-col block
              (host-interleaved [256 gate|256 up]) -> silu+mult drains bank
  actT = PE-transpose(act);  y = actT.T @ dnT;  top-8 chain on DVE
  (pinned after casts);  out = w*y -> DRAM
"""
import numpy as np

import concourse.bass as bass
import concourse.mybir as mybir
import concourse.tile as tile
from concourse import bacc
from concourse.bass_utils import run_bass_kernel_spmd
from concourse.masks import make_identity

T = 2048
H = 2048
I2 = 1536
I = 768
E = 32
G = 8
TOP_K = 8
KO = H // 128
JO = I // 128
TCH = 128
NCHUNK = T // TCH
HB = 512
NEG_BIG = -1.0e9

F32 = mybir.dt.float32
F32R = mybir.dt.float32r
U8 = mybir.dt.uint8
AX = mybir.AxisListType.X
OP = mybir.AluOpType
ACTF = mybir.ActivationFunctionType

_CACHED_NC = None


def _build():
    global _CACHED_NC
    if _CACHED_NC is not None:
        return _CACHED_NC
    nc = bacc.Bacc("TRN2", target_bir_lowering=False, debug=False, num_devices=G)

    xT_d = nc.dram_tensor("xT", [H, T], F32R, kind="ExternalInput")
    gu_d = nc.dram_tensor("gu", [H, I2], F32R, kind="ExternalInput")
    gw_d = nc.dram_tensor("gw", [H, E], F32, kind="ExternalInput")
    dnT_d = nc.dram_tensor("dnT", [I, H], F32R, kind="ExternalInput")
    mgb_d = nc.dram_tensor("mgb", [128, E], F32, kind="ExternalInput")
    y_d = nc.dram_tensor("y", [T, H], F32, kind="ExternalOutput")

    xT_ap = xT_d.ap().rearrange("(ko p) t -> p ko t", p=128)
    xT_ap32 = xT_d.ap().bitcast(F32).rearrange("(ko p) t -> p ko t", p=128)
    gu_ap = gu_d.ap().rearrange("(ko p) o -> p ko o", p=128)
    gw_ap = gw_d.ap().rearrange("(ko p) e -> p ko e", p=128)
    dnT_ap = dnT_d.ap().rearrange("(jo p) h -> p jo h", p=128)

    with tile.TileContext(nc) as tc:
        with (
            tc.tile_pool(name="const", bufs=1) as cpool,
            tc.tile_pool(name="weights", bufs=1) as wpool,
            tc.tile_pool(name="xin", bufs=2) as xpool,
            tc.tile_pool(name="xrin", bufs=2) as xrpool,
            tc.tile_pool(name="acts", bufs=1) as apool,
            tc.tile_pool(name="router", bufs=2) as rpool,
            tc.tile_pool(name="yout", bufs=2) as ypool,
            tc.tile_pool(name="plg", bufs=1, space="PSUM") as plg,
            tc.tile_pool(name="ph", bufs=3, space="PSUM") as pph,
            tc.tile_pool(name="ps", bufs=1, space="PSUM") as pps,
            tc.tile_pool(name="py", bufs=2, space="PSUM") as ppy,
        ):
            identity = cpool.tile([128, 128], F32, tag="identity")
            make_identity(nc, identity)
            negbig = cpool.tile([128, E], F32, tag="negbig")
            nc.vector.memset(negbig, NEG_BIG)
            mgb_sb = cpool.tile([128, E], F32, tag="mgb")
            nc.sync.dma_start(mgb_sb[:], mgb_d.ap())
            gw_sb = cpool.tile([128, KO, E], F32, tag="gw")
            nc.sync.dma_start(gw_sb[:], gw_ap)

            gu_sb = wpool.tile([128, KO, I2], F32R, tag="gu")
            dn_sb = wpool.tile([128, JO, H], F32R, tag="dn")

            xtiles = {}
            xrtiles = {}

            def load_chunk(ci):
                t = xpool.tile([128, KO, TCH], F32R, tag="xT_c",
                               name=f"xT_c{ci}")
                nc.sync.dma_start(t[:], xT_ap[:, :, ci * TCH:(ci + 1) * TCH])
                xtiles[ci] = t

            def load_xr(ci):
                t = xrpool.tile([128, KO, TCH], F32, tag="xr",
                                name=f"xr{ci}")
                nc.sync.dma_start(t[:], xT_ap32[:, :, ci * TCH:(ci + 1) * TCH])
                xrtiles[ci] = t

            load_xr(0)
            load_chunk(0)
            nc.sync.dma_start(gu_sb[:, :, 0:HB], gu_ap[:, :, 0:HB])

            for tci in range(NCHUNK):
                tsl = slice(tci * TCH, (tci + 1) * TCH)
                if tci + 1 < NCHUNK:
                    load_xr(tci + 1)
                    load_chunk(tci + 1)
                xT_c = xtiles.pop(tci)
                xr_c = xrtiles.pop(tci)

                # ---- router logits (exact fp32), gw stationary ----
                lg_ps = plg.tile([E, TCH], F32, tag="lg_ps")
                for k in range(KO):
                    nc.tensor.matmul(
                        lg_ps[:], gw_sb[:, k], xr_c[:, k],
                        start=(k == 0), stop=(k == KO - 1),
                    )
                lgT_sb = rpool.tile([E, TCH], F32, tag="lgT_sb")
                nc.vector.tensor_copy(lgT_sb[:], lg_ps[:])
                s_ps = pps.tile([128, JO + 1, TCH], F32, tag="s_ps")
                nc.tensor.transpose(s_ps[:, JO, :E], lgT_sb[:],
                                    identity[:E, :E])
                logits = rpool.tile([128, E], F32, tag="logits")
                nc.vector.tensor_copy(logits[:], s_ps[:, JO, :E])

                if tci == 0:
                    nc.sync.dma_start(gu_sb[:, :, HB:2 * HB],
                                      gu_ap[:, :, HB:2 * HB])

                # ---- M1, b-major: one PSUM bank at a time ----
                act_sb = apool.tile([128, I], F32, tag="act")
                for b in range(3):
                    h_ps = pph.tile([128, HB], F32, tag="h_ps",
                                    name=f"h{tci}_{b}")
                    for k in range(KO):
                        nc.tensor.matmul(
                            h_ps[:], xT_c[:, k],
                            gu_sb[:, k, b * HB:(b + 1) * HB],
                            start=(k == 0), stop=(k == KO - 1),
                        )
                    if tci == 0 and b == 0:
                        nc.sync.dma_start(gu_sb[:, :, 2 * HB:I2],
                                          gu_ap[:, :, 2 * HB:I2])
                    if tci == 0 and b == 1:
                        for j in range(JO):
                            nc.sync.dma_start(dn_sb[:, j], dnT_ap[:, j])
                    # host interleave: h_b = [256 gate | 256 up]
                    silu_sb = apool.tile([128, 256], F32, tag="silu")
                    nc.scalar.activation(silu_sb[:], h_ps[:, :256],
                                         ACTF.Silu)
                    nc.vector.tensor_tensor(
                        act_sb[:, 256 * b:256 * (b + 1)], silu_sb[:],
                        h_ps[:, 256:], OP.mult,
                    )

                # ---- transpose act -> actT ----
                actT_sb = apool.tile([128, JO, TCH], F32R, tag="actT")
                for j in range(JO):
                    nc.tensor.transpose(
                        s_ps[:, j], act_sb[:, j * 128:(j + 1) * 128],
                        identity,
                    )
                    nc.vector.tensor_copy(actT_sb[:, j], s_ps[:, j])

                # ---- M2 ----
                y_pss = []
                for hb in range(H // HB):
                    y_ps = ppy.tile([128, HB], F32, tag="y_ps",
                                    name=f"y_ps{tci}_{hb}")
                    for j in range(JO):
                        nc.tensor.matmul(
                            y_ps[:], actT_sb[:, j],
                            dn_sb[:, j, hb * HB:(hb + 1) * HB],
                            start=(j == 0), stop=(j == JO - 1),
                        )
                    y_pss.append(y_ps)

                # ---- top-8 router chain (DVE) ----
                # Pin the chain after the casts so it can't hog DVE while the
                # next chunk's SwiGLU needs the h banks released. The first
                # chunk has an idle DVE (DMA-bound head) and the last has no
                # successor to protect, so let those chains run early.
                cur = rpool.tile([128, E], F32, tag="cur")
                if 0 < tci < NCHUNK - 1:
                    dep = rpool.tile([128, E], F32, tag="dep")
                    nc.vector.tensor_scalar(
                        dep[:], actT_sb[:, JO - 1, :E].bitcast(F32), 0.0,
                        None, OP.mult)
                    nc.vector.tensor_tensor(cur[:], logits[:], dep[:], OP.add)
                else:
                    nc.vector.tensor_copy(cur[:], logits[:])
                msk = rpool.tile([128, E], U8, tag="msk")
                m1 = rpool.tile([128, 1], F32, tag="m1")
                mk = rpool.tile([128, 1], F32, tag="mk")
                for it in range(TOP_K - 1):
                    tgt = m1 if it == 0 else mk
                    nc.vector.reduce_max(tgt[:], cur[:], axis=AX)
                    nc.vector.tensor_scalar(msk[:], cur[:], tgt[:],
                                            None, OP.is_ge)
                    nc.vector.copy_predicated(cur[:], msk[:], negbig[:])
                m8 = rpool.tile([128, 1], F32, tag="m8")
                nc.vector.reduce_max(m8[:], cur[:], axis=AX)

                nm1 = rpool.tile([128, 1], F32, tag="nm1")
                nc.vector.tensor_scalar(nm1[:], m1[:], -1.0, None, OP.mult)
                mask8 = rpool.tile([128, E], F32, tag="mask8")
                nc.vector.tensor_scalar(mask8[:], logits[:], m8[:],
                                        None, OP.is_ge)
                ew = rpool.tile([128, E], F32, tag="ew")
                nc.scalar.activation(ew[:], logits[:], ACTF.Exp, bias=nm1[:])
                nc.vector.tensor_tensor(ew[:], ew[:], mask8[:], OP.mult)
                s8 = rpool.tile([128, 1], F32, tag="s8")
                nc.vector.reduce_sum(s8[:], ew[:], axis=AX)
                nc.vector.tensor_tensor(ew[:], ew[:], mgb_sb[:], OP.mult)
                num = rpool.tile([128, 1], F32, tag="num")
                nc.vector.reduce_sum(num[:], ew[:], axis=AX)
                rs = rpool.tile([128, 1], F32, tag="rs")
                nc.vector.reciprocal(rs[:], s8[:])
                w_t = rpool.tile([128, 1], F32, tag="w_t")
                nc.vector.tensor_tensor(w_t[:], num[:], rs[:], OP.mult)

                # ---- scale + store ----
                for hb in range(H // HB):
                    y_sb = ypool.tile([128, HB], F32, tag="y_sb")
                    nc.vector.tensor_scalar(
                        y_sb[:], y_pss[hb][:], w_t[:], None, OP.mult,
                    )
                    nc.sync.dma_start(
                        y_d.ap()[tsl, hb * HB:(hb + 1) * HB], y_sb[:],
                    )
    nc.compile()
    _CACHED_NC = nc
    return nc


_GATEUP_PERM = np.concatenate(
    [np.r_[256 * b:256 * b + 256, 768 + 256 * b:768 + 256 * b + 256]
     for b in range(3)]
)


def prepare_in_maps(hidden_states, gate_weight, gate_up_proj, down_proj,
                    merge_groups, dominant_experts):
    x = np.asarray(hidden_states, dtype=np.float32).reshape(T, H)
    xT = np.ascontiguousarray(x.T)
    gw = np.asarray(gate_weight, dtype=np.float32)
    gwT = np.ascontiguousarray(gw.T)
    mg = np.asarray(merge_groups).astype(np.int64)
    de = np.asarray(dominant_experts).astype(np.int64)
    gup = np.asarray(gate_up_proj, dtype=np.float32)
    dnp_ = np.asarray(down_proj, dtype=np.float32)

    in_maps = []
    for g in range(G):
        e = int(de[g])
        guT = np.ascontiguousarray(gup[e].T[:, _GATEUP_PERM])
        dnT = np.ascontiguousarray(dnp_[e].T)
        mgb = np.ascontiguousarray(
            np.broadcast_to((mg == g).astype(np.float32)[None, :], (128, E))
        )
        in_maps.append({"xT": xT, "gu": guT, "gw": gwT, "dnT": dnT,
                        "mgb": mgb})
    return in_maps


def kernel(hidden_states, gate_weight, gate_up_proj, down_proj,
           merge_groups, dominant_experts):
    in_maps = prepare_in_maps(hidden_states, gate_weight, gate_up_proj,
                              down_proj, merge_groups, dominant_experts)
    nc = _build()
    res = run_bass_kernel_spmd(nc, in_maps, core_ids=list(range(G)),
                               trace=False)
    out = np.zeros((T, H), dtype=np.float64)
    for r in res.results:
        out += r["y"].astype(np.float64)
    return out.astype(np.float32).reshape(1, T, H)



# revision 9
# speedup vs baseline: 1.5285x; 1.5285x over previous
"""TRN2 Bass kernel for nn_HCSMoEQwen3MoeSparseMoeBlock (8-core, load-balanced
expert-parallel with capacity slots).

Observation: only ~66% of (token, group) pairs are active (a token
contributes to group g only if one of its top-8 experts merges into g), but
per-group token counts are skewed, so plain expert-parallel (core g = group
g, all tokens) wastes ~34% of the FLOPs. Instead:

- Host computes the routing membership (exact fp32, redundant copy of the
  router purely for data placement) and packs per-group token segments into
  8 cores x 3 uniform weight "slots" of (s0,s1,s2) 128-token chunks
  (DFS packer, ladder of slot shapes; (4,4,3) for the staged data = 1408
  tokens/core vs 2048).
- Device (same program on all 8 cores): for each slot, stream that slot's
  expert weights (bf16, double-buffered); router logits in fp32r on
  256-token batches (ap>=256 keeps the PE at 1 cycle/row; the separate
  fp32 x stream exists only for the router); M1/SwiGLU/transpose/M2 in
  bf16 (1 cycle/row, halved SBUF+DMA); on-device top-8 + group weight;
  w*y stored per chunk.
- Host scatter-adds each slot's rows into the full [T, H] output.

All routing math that affects output VALUES runs on device; the host router
only decides which (token, group) pairs each core computes.
"""
import numpy as np
import ml_dtypes

import concourse.bass as bass
import concourse.mybir as mybir
import concourse.tile as tile
from concourse import bacc
from concourse.bass_utils import run_bass_kernel_spmd
from concourse.masks import make_identity

T = 2048
H = 2048
I2 = 1536
I = 768
E = 32
G = 8
NSLOT = 3
TOP_K = 8
KO = H // 128
JO = I // 128
TCH = 128
HB = 512
NEG_BIG = -1.0e9
BF16NP = ml_dtypes.bfloat16

F32 = mybir.dt.float32
F32R = mybir.dt.float32r
BF16 = mybir.dt.bfloat16
U8 = mybir.dt.uint8
AX = mybir.AxisListType.X
OP = mybir.AluOpType
ACTF = mybir.ActivationFunctionType

_CACHED_NC = {}

# Slot-shape ladder: uniform (s0,s1,s2) chunks per core, 8 slots per
# position. First feasible entry wins; (16,16,16) always fits (any group
# needs at most ceil(2048/128)=16 chunks).
_LADDER = [
    (4, 4, 3), (4, 4, 4), (5, 4, 4), (5, 5, 4), (5, 5, 5), (6, 5, 5),
    (6, 6, 5), (6, 6, 6), (7, 7, 6), (7, 7, 7), (8, 8, 7), (8, 8, 8),
    (10, 10, 10), (12, 12, 12), (14, 14, 14), (16, 16, 16),
]


def _dfs(order, i, avail, nchunks):
    """Assign each group a multiset of slot sizes covering its chunk count.

    avail: dict size -> remaining slot count. Returns {group: [sizes]}.
    """
    if i == len(order):
        return {}
    g = order[i]
    n = nchunks[g]
    if n == 0:
        rest = _dfs(order, i + 1, avail, nchunks)
        if rest is None:
            return None
        rest[g] = []
        return rest
    vs = sorted(avail.keys(), reverse=True)
    combos = []

    def gen(idx, covered, cur):
        if covered >= n:
            combos.append((covered - n, len(cur), list(cur)))
            return
        if idx == len(vs):
            return
        v = vs[idx]
        top = min(avail[v] - cur.count(v), (n - covered + v - 1) // v)
        for k in range(top, -1, -1):
            gen(idx + 1, covered + k * v, cur + [v] * k)

    gen(0, 0, [])
    combos.sort(key=lambda c: (c[0], c[1]))
    for _, _, sizes in combos:
        for v in sizes:
            avail[v] -= 1
        rest = _dfs(order, i + 1, avail, nchunks)
        if rest is not None:
            rest[g] = sizes
            return rest
        for v in sizes:
            avail[v] += 1
    return None


def _pack(nchunks):
    for sizes in _LADDER:
        avail = {}
        for v in sizes:
            avail[v] = avail.get(v, 0) + 8
        order = sorted(range(G), key=lambda g: -nchunks[g])
        res = _dfs(order, 0, dict(avail), nchunks)
        if res is not None:
            return sizes, res
    raise RuntimeError("slot packing failed")


def _build(sizes):
    if sizes in _CACHED_NC:
        return _CACHED_NC[sizes]
    c_chunks = sum(sizes)
    C = c_chunks * TCH
    # 256-token router batches over the whole token range
    batches = []
    c = 0
    while c < c_chunks:
        n = min(2, c_chunks - c)
        batches.append((c, n))
        c += n
    chunk_slot = []
    for s, v in enumerate(sizes):
        chunk_slot += [s] * v
    slot_start = [0]
    for v in sizes[:-1]:
        slot_start.append(slot_start[-1] + v)
    # prefetch slot s one chunk into slot s-1
    prefetch_chunk = {slot_start[s - 1] + 1: s for s in range(1, NSLOT)}

    nc = bacc.Bacc("TRN2", target_bir_lowering=False, debug=False,
                   num_devices=G)

    x32_d = nc.dram_tensor("x32", [H, C], F32R, kind="ExternalInput")
    x16_d = nc.dram_tensor("x16", [H, C], BF16, kind="ExternalInput")
    gu_d = nc.dram_tensor("gu", [H, NSLOT * I2], BF16, kind="ExternalInput")
    dn_d = nc.dram_tensor("dn", [I, NSLOT * H], BF16, kind="ExternalInput")
    gw_d = nc.dram_tensor("gw", [H, E], F32R, kind="ExternalInput")
    mgb_d = nc.dram_tensor("mgb", [128, NSLOT * E], F32, kind="ExternalInput")
    y_d = nc.dram_tensor("y", [C, H], F32, kind="ExternalOutput")

    x32_ap = x32_d.ap().rearrange("(ko p) t -> p ko t", p=128)
    x16_ap = x16_d.ap().rearrange("(ko p) t -> p ko t", p=128)
    gu_ap = gu_d.ap().rearrange("(ko p) o -> p ko o", p=128)
    dn_ap = dn_d.ap().rearrange("(jo p) h -> p jo h", p=128)
    gw_ap = gw_d.ap().rearrange("(ko p) e -> p ko e", p=128)
    mgb_ap = mgb_d.ap().rearrange("p (s e) -> p s e", e=E)

    with tile.TileContext(nc) as tc:
        with (
            tc.tile_pool(name="const", bufs=1) as cpool,
            tc.tile_pool(name="weights", bufs=2) as wpool,
            tc.tile_pool(name="x32p", bufs=1) as xpool,
            tc.tile_pool(name="x16p", bufs=2) as x16pool,
            tc.tile_pool(name="acts", bufs=2) as apool,
            tc.tile_pool(name="actT", bufs=2) as atpool,
            tc.tile_pool(name="router", bufs=2) as rpool,
            tc.tile_pool(name="yout", bufs=2) as ypool,
            tc.tile_pool(name="plg", bufs=1, space="PSUM") as plg,
            tc.tile_pool(name="plt", bufs=1, space="PSUM") as plt,
            tc.tile_pool(name="ph", bufs=3, space="PSUM") as pph,
            tc.tile_pool(name="ps", bufs=1, space="PSUM") as pps,
            tc.tile_pool(name="py", bufs=2, space="PSUM") as ppy,
        ):
            id_f32 = cpool.tile([128, 128], F32, tag="id_f32")
            make_identity(nc, id_f32)
            id_bf = cpool.tile([128, 128], BF16, tag="id_bf")
            nc.vector.tensor_copy(id_bf[:], id_f32[:])
            id_r = cpool.tile([128, E], F32R, tag="id_r")
            nc.vector.tensor_copy(id_r[:], id_f32[:, :E])
            negbig = cpool.tile([128, E], F32, tag="negbig")
            nc.vector.memset(negbig, NEG_BIG)
            mgb_sb = cpool.tile([128, NSLOT, E], F32, tag="mgb")
            nc.sync.dma_start(mgb_sb[:], mgb_ap)
            gw_sb = cpool.tile([128, KO, E], F32R, tag="gw")
            nc.sync.dma_start(gw_sb[:], gw_ap)

            xb32 = {}
            xb16 = {}

            def load_batch(bi):
                c0, n = batches[bi]
                w = n * TCH
                t32 = xpool.tile([128, KO, 256], F32R, tag="x32",
                                 name=f"x32_{bi}")
                nc.sync.dma_start(t32[:, :, :w],
                                  x32_ap[:, :, c0 * TCH:c0 * TCH + w])
                t16 = x16pool.tile([128, KO, 256], BF16, tag="x16",
                                   name=f"x16_{bi}")
                nc.sync.dma_start(t16[:, :, :w],
                                  x16_ap[:, :, c0 * TCH:c0 * TCH + w])
                xb32[bi] = t32
                xb16[bi] = t16

            gu_tiles = {}
            dn_tiles = {}

            def load_slot(s):
                gu_t = wpool.tile([128, KO, I2], BF16, tag="gu",
                                  name=f"gu{s}")
                for p in range(3):
                    nc.sync.dma_start(
                        gu_t[:, :, p * HB:(p + 1) * HB],
                        gu_ap[:, :, s * I2 + p * HB:s * I2 + (p + 1) * HB],
                    )
                dn_t = wpool.tile([128, JO, H], BF16, tag="dn",
                                  name=f"dn{s}")
                nc.sync.dma_start(dn_t[:], dn_ap[:, :, s * H:(s + 1) * H])
                gu_tiles[s] = gu_t
                dn_tiles[s] = dn_t

            load_batch(0)
            load_slot(0)

            for bi, (c0, nch) in enumerate(batches):
                t32 = xb32.pop(bi)
                t16 = xb16.pop(bi)
                w = nch * TCH

                # ---- router logits for the batch (fp32r, ap>=256) ----
                lg = plg.tile([E, 256], F32, tag="lg", name=f"lg{bi}")
                for k in range(KO):
                    nc.tensor.matmul(lg[:, :w], gw_sb[:, k], t32[:, k, :w],
                                     start=(k == 0), stop=(k == KO - 1))
                lgT = rpool.tile([E, 256], F32R, tag="lgT", name=f"lgT{bi}")
                nc.vector.tensor_copy(lgT[:, :w], lg[:, :w])
                lgt_ps = plt.tile([128, 2, E], F32R, tag="lgt",
                                  name=f"lgt{bi}")
                for hh in range(nch):
                    nc.tensor.transpose(lgt_ps[:, hh],
                                        lgT[:, hh * TCH:(hh + 1) * TCH],
                                        id_r[:E, :E])

                if bi + 1 < len(batches):
                    load_batch(bi + 1)

                for hh in range(nch):
                    gc = c0 + hh
                    s = chunk_slot[gc]
                    if gc in prefetch_chunk:
                        load_slot(prefetch_chunk[gc])
                    gu_t = gu_tiles[s]
                    dn_t = dn_tiles[s]

                    logits = rpool.tile([128, E], F32, tag="logits",
                                        name=f"logits{gc}")
                    nc.vector.tensor_copy(logits[:],
                                          lgt_ps[:, hh].bitcast(F32))

                    # ---- top-8 chain (DVE) issued early: runs during M1
                    cur = rpool.tile([128, E], F32, tag="cur",
                                     name=f"cur{gc}")
                    nc.vector.tensor_copy(cur[:], logits[:])
                    msk = rpool.tile([128, E], U8, tag="msk")
                    m1 = rpool.tile([128, 1], F32, tag="m1")
                    mk = rpool.tile([128, 1], F32, tag="mk")
                    for it in range(TOP_K - 1):
                        tgt = m1 if it == 0 else mk
                        nc.vector.reduce_max(tgt[:], cur[:], axis=AX)
                        nc.vector.tensor_scalar(msk[:], cur[:], tgt[:],
                                                None, OP.is_ge)
                        nc.vector.copy_predicated(cur[:], msk[:], negbig[:])
                    m8 = rpool.tile([128, 1], F32, tag="m8")
                    nc.vector.reduce_max(m8[:], cur[:], axis=AX)
                    nm1 = rpool.tile([128, 1], F32, tag="nm1")
                    nc.vector.tensor_scalar(nm1[:], m1[:], -1.0, None,
                                            OP.mult)
                    mask8 = rpool.tile([128, E], F32, tag="mask8")
                    nc.vector.tensor_scalar(mask8[:], logits[:], m8[:],
                                            None, OP.is_ge)
                    ew = rpool.tile([128, E], F32, tag="ew")
                    nc.scalar.activation(ew[:], logits[:], ACTF.Exp,
                                         bias=nm1[:])
                    nc.vector.tensor_tensor(ew[:], ew[:], mask8[:], OP.mult)
                    s8 = rpool.tile([128, 1], F32, tag="s8")
                    nc.vector.reduce_sum(s8[:], ew[:], axis=AX)
                    nc.vector.tensor_tensor(ew[:], ew[:], mgb_sb[:, s],
                                            OP.mult)
                    num = rpool.tile([128, 1], F32, tag="num")
                    nc.vector.reduce_sum(num[:], ew[:], axis=AX)
                    rs = rpool.tile([128, 1], F32, tag="rs")
                    nc.vector.reciprocal(rs[:], s8[:])
                    w_t = rpool.tile([128, 1], F32, tag="w_t")
                    nc.vector.tensor_tensor(w_t[:], num[:], rs[:], OP.mult)

                    # ---- M1 (bf16): h = x16_chunk.T @ gu, b-major ----
                    act_sb = apool.tile([128, I], BF16, tag="act",
                                        name=f"act{gc}")
                    for b in range(3):
                        h_ps = pph.tile([128, HB], F32, tag="h_ps",
                                        name=f"h{gc}_{b}")
                        for k in range(KO):
                            nc.tensor.matmul(
                                h_ps[:],
                                t16[:, k, hh * TCH:(hh + 1) * TCH],
                                gu_t[:, k, b * HB:(b + 1) * HB],
                                start=(k == 0), stop=(k == KO - 1),
                            )
                        # host interleave: each 512-col block = [256 gate|256 up]
                        silu_sb = apool.tile([128, 256], F32, tag="silu",
                                             name=f"silu{gc}_{b}")
                        nc.scalar.activation(silu_sb[:], h_ps[:, :256],
                                             ACTF.Silu)
                        nc.vector.tensor_tensor(
                            act_sb[:, 256 * b:256 * (b + 1)], silu_sb[:],
                            h_ps[:, 256:], OP.mult,
                        )

                    # ---- transpose act -> actT (bf16) ----
                    actT = atpool.tile([128, JO, TCH], BF16, tag="actT",
                                       name=f"actT{gc}")
                    s_ps = pps.tile([128, JO, TCH], BF16, tag="s_ps",
                                    name=f"sps{gc}")
                    for j in range(JO):
                        nc.tensor.transpose(s_ps[:, j],
                                            act_sb[:, j * 128:(j + 1) * 128],
                                            id_bf)
                        nc.vector.tensor_copy(actT[:, j], s_ps[:, j])

                    # ---- M2 (bf16) ----
                    y_pss = []
                    for hb in range(H // HB):
                        y_ps = ppy.tile([128, HB], F32, tag="y_ps",
                                        name=f"y{gc}_{hb}")
                        for j in range(JO):
                            nc.tensor.matmul(
                                y_ps[:], actT[:, j],
                                dn_t[:, j, hb * HB:(hb + 1) * HB],
                                start=(j == 0), stop=(j == JO - 1),
                            )
                        y_pss.append(y_ps)

                    # ---- scale + store ----
                    for hb in range(H // HB):
                        y_sb = ypool.tile([128, HB], F32, tag="y_sb",
                                          name=f"ysb{gc}_{hb}")
                        nc.vector.tensor_scalar(y_sb[:], y_pss[hb][:],
                                                w_t[:], None, OP.mult)
                        nc.sync.dma_start(
                            y_d.ap()[gc * TCH:(gc + 1) * TCH,
                                     hb * HB:(hb + 1) * HB], y_sb[:],
                        )
    nc.compile()
    _CACHED_NC[sizes] = nc
    return nc


_GATEUP_PERM = np.concatenate(
    [np.r_[256 * b:256 * b + 256, 768 + 256 * b:768 + 256 * b + 256]
     for b in range(3)]
)


def prepare(hidden_states, gate_weight, gate_up_proj, down_proj,
            merge_groups, dominant_experts):
    x = np.asarray(hidden_states, dtype=np.float32).reshape(T, H)
    gw = np.asarray(gate_weight, dtype=np.float32)
    mg = np.asarray(merge_groups).astype(np.int64)
    de = np.asarray(dominant_experts).astype(np.int64)
    gup = np.asarray(gate_up_proj, dtype=np.float32)
    dnp_ = np.asarray(down_proj, dtype=np.float32)

    # Host routing (membership only; device recomputes the weights)
    logits = x @ gw.T
    top8 = np.argpartition(-logits, TOP_K - 1, axis=1)[:, :TOP_K]
    active = np.zeros((T, G), np.bool_)
    active[np.arange(T)[:, None], mg[top8]] = True
    idx_lists = [np.nonzero(active[:, g])[0] for g in range(G)]
    nchunks = [(len(ix) + TCH - 1) // TCH for ix in idx_lists]

    sizes, segcaps = _pack(nchunks)
    c_chunks = sum(sizes)
    C = c_chunks * TCH

    # materialize segments grouped by size class
    segs_by_size = {}
    for g in range(G):
        pos = 0
        for scap in sorted(segcaps[g], reverse=True):
            tk = idx_lists[g][pos:pos + scap * TCH]
            pos += len(tk)
            segs_by_size.setdefault(scap, []).append((g, tk))
        assert pos >= len(idx_lists[g])

    core_slots = [[(None, np.zeros(0, np.int64))] * NSLOT for _ in range(G)]
    for p, v in enumerate(sizes):
        pool = segs_by_size.get(v, [])
        for c in range(G):
            if pool:
                core_slots[c][p] = pool.pop(0)
    for v, pool in segs_by_size.items():
        assert not pool, f"unplaced segments of size {v}"

    xT = np.ascontiguousarray(x.T)
    gwT = np.ascontiguousarray(gw.T)
    guT16 = {}
    dnT16 = {}
    for g in range(G):
        e = int(de[g])
        if e not in guT16:
            guT16[e] = np.ascontiguousarray(
                gup[e].T[:, _GATEUP_PERM]).astype(BF16NP)
            dnT16[e] = np.ascontiguousarray(dnp_[e].T).astype(BF16NP)

    slot_off = [0]
    for v in sizes:
        slot_off.append(slot_off[-1] + v)

    in_maps = []
    scatter = []
    for c in range(G):
        tok = np.zeros(C, np.int64)
        mgb = np.zeros((128, NSLOT * E), np.float32)
        gu_cat = np.zeros((H, NSLOT * I2), BF16NP)
        dn_cat = np.zeros((I, NSLOT * H), BF16NP)
        sc = []
        for p in range(NSLOT):
            g, tk = core_slots[c][p]
            off = slot_off[p] * TCH
            if g is None or len(tk) == 0:
                continue
            tok[off:off + len(tk)] = tk
            mgb[:, p * E:(p + 1) * E] = (mg == g).astype(np.float32)[None, :]
            e = int(de[g])
            gu_cat[:, p * I2:(p + 1) * I2] = guT16[e]
            dn_cat[:, p * H:(p + 1) * H] = dnT16[e]
            sc.append((tk, off))
        x32c = np.ascontiguousarray(xT[:, tok])
        in_maps.append({
            "x32": x32c,
            "x16": x32c.astype(BF16NP),
            "gu": gu_cat,
            "dn": dn_cat,
            "gw": gwT,
            "mgb": mgb,
        })
        scatter.append(sc)
    return sizes, in_maps, scatter


def kernel(hidden_states, gate_weight, gate_up_proj, down_proj,
           merge_groups, dominant_experts):
    sizes, in_maps, scatter = prepare(
        hidden_states, gate_weight, gate_up_proj, down_proj,
        merge_groups, dominant_experts)
    nc = _build(sizes)
    res = run_bass_kernel_spmd(nc, in_maps, core_ids=list(range(G)),
                               trace=False)
    out = np.zeros((T, H), dtype=np.float64)
    for c, r in enumerate(res.results):
        y = r["y"].astype(np.float64)
        for tk, off in scatter[c]:
            out[tk] += y[off:off + len(tk)]
    return out.astype(np.float32).reshape(1, T, H)


# revision 15
# speedup vs baseline: 1.7424x; 1.1399x over previous
"""TRN2 Bass kernel for nn_HCSMoEQwen3MoeSparseMoeBlock (8-core, load-balanced
expert-parallel with capacity slots).

Observation: only ~66% of (token, group) pairs are active (a token
contributes to group g only if one of its top-8 experts merges into g), but
per-group token counts are skewed, so plain expert-parallel (core g = group
g, all tokens) wastes ~34% of the FLOPs. Instead:

- Host computes the routing membership (exact fp32, redundant copy of the
  router purely for data placement) and packs per-group token segments into
  8 cores x 3 uniform weight "slots" of (s0,s1,s2) 128-token chunks
  (DFS packer, ladder of slot shapes; (4,4,3) for the staged data = 1408
  tokens/core vs 2048).
- Device (same program on all 8 cores): for each slot, stream that slot's
  expert weights (bf16, double-buffered); router logits in fp32r on
  256-token batches (ap>=256 keeps the PE at 1 cycle/row; the separate
  fp32 x stream exists only for the router); M1/SwiGLU/transpose/M2 in
  bf16 (1 cycle/row, halved SBUF+DMA); on-device top-8 + group weight;
  w*y stored per chunk.
- Host scatter-adds each slot's rows into the full [T, H] output.

All routing math that affects output VALUES runs on device; the host router
only decides which (token, group) pairs each core computes.
"""
import numpy as np
import ml_dtypes

import concourse.bass as bass
import concourse.mybir as mybir
import concourse.tile as tile
from concourse import bacc
from concourse.bass_utils import run_bass_kernel_spmd
from concourse.masks import make_identity

T = 2048
H = 2048
I2 = 1536
I = 768
E = 32
G = 8
NSLOT = 3
TOP_K = 8
KO = H // 128
JO = I // 128
TCH = 128
HB = 512
NEG_BIG = -1.0e9
BF16NP = ml_dtypes.bfloat16

F32 = mybir.dt.float32
F32R = mybir.dt.float32r
BF16 = mybir.dt.bfloat16
U8 = mybir.dt.uint8
AX = mybir.AxisListType.X
OP = mybir.AluOpType
ACTF = mybir.ActivationFunctionType

_CACHED_NC = {}

# Slot-shape ladder: uniform (s0,s1,s2) chunks per core, 8 slots per
# position. First feasible entry wins; (16,16,16) always fits (any group
# needs at most ceil(2048/128)=16 chunks).
_LADDER = [
    (4, 4, 3), (4, 4, 4), (5, 4, 4), (5, 5, 4), (5, 5, 5), (6, 5, 5),
    (6, 6, 5), (6, 6, 6), (7, 7, 6), (7, 7, 7), (8, 8, 7), (8, 8, 8),
    (10, 10, 10), (12, 12, 12), (14, 14, 14), (16, 16, 16),
]


def _dfs(order, i, avail, nchunks):
    """Assign each group a multiset of slot sizes covering its chunk count.

    avail: dict size -> remaining slot count. Returns {group: [sizes]}.
    """
    if i == len(order):
        return {}
    g = order[i]
    n = nchunks[g]
    if n == 0:
        rest = _dfs(order, i + 1, avail, nchunks)
        if rest is None:
            return None
        rest[g] = []
        return rest
    vs = sorted(avail.keys(), reverse=True)
    combos = []

    def gen(idx, covered, cur):
        if covered >= n:
            combos.append((covered - n, len(cur), list(cur)))
            return
        if idx == len(vs):
            return
        v = vs[idx]
        top = min(avail[v] - cur.count(v), (n - covered + v - 1) // v)
        for k in range(top, -1, -1):
            gen(idx + 1, covered + k * v, cur + [v] * k)

    gen(0, 0, [])
    combos.sort(key=lambda c: (c[0], c[1]))
    for _, _, sizes in combos:
        for v in sizes:
            avail[v] -= 1
        rest = _dfs(order, i + 1, avail, nchunks)
        if rest is not None:
            rest[g] = sizes
            return rest
        for v in sizes:
            avail[v] += 1
    return None


def _pack(nchunks):
    for sizes in _LADDER:
        avail = {}
        for v in sizes:
            avail[v] = avail.get(v, 0) + 8
        order = sorted(range(G), key=lambda g: -nchunks[g])
        res = _dfs(order, 0, dict(avail), nchunks)
        if res is not None:
            return sizes, res
    raise RuntimeError("slot packing failed")


def _build(sizes):
    if sizes in _CACHED_NC:
        return _CACHED_NC[sizes]
    c_chunks = sum(sizes)
    C = c_chunks * TCH
    # 256-token router batches over the whole token range
    batches = []
    c = 0
    while c < c_chunks:
        n = min(2, c_chunks - c)
        batches.append((c, n))
        c += n
    chunk_slot = []
    for s, v in enumerate(sizes):
        chunk_slot += [s] * v
    slot_start = [0]
    for v in sizes[:-1]:
        slot_start.append(slot_start[-1] + v)
    # prefetch slot s one chunk into slot s-1
    prefetch_chunk = {slot_start[s - 1] + 1: s for s in range(1, NSLOT)}

    nc = bacc.Bacc("TRN2", target_bir_lowering=False, debug=False,
                   num_devices=G)

    x32_d = nc.dram_tensor("x32", [H, C], F32R, kind="ExternalInput")
    x16_d = nc.dram_tensor("x16", [H, C], BF16, kind="ExternalInput")
    gu_d = nc.dram_tensor("gu", [H, NSLOT * I2], BF16, kind="ExternalInput")
    dn_d = nc.dram_tensor("dn", [I, NSLOT * H], BF16, kind="ExternalInput")
    gw_d = nc.dram_tensor("gw", [H, E], F32R, kind="ExternalInput")
    mgb_d = nc.dram_tensor("mgb", [128, NSLOT * E], F32, kind="ExternalInput")
    y_d = nc.dram_tensor("y", [C, H], F32, kind="ExternalOutput")

    x32_ap = x32_d.ap().rearrange("(ko p) t -> p ko t", p=128)
    x16_ap = x16_d.ap().rearrange("(ko p) t -> p ko t", p=128)
    gu_ap = gu_d.ap().rearrange("(ko p) o -> p ko o", p=128)
    dn_ap = dn_d.ap().rearrange("(jo p) h -> p jo h", p=128)
    gw_ap = gw_d.ap().rearrange("(ko p) e -> p ko e", p=128)
    mgb_ap = mgb_d.ap().rearrange("p (s e) -> p s e", e=E)

    with tile.TileContext(nc) as tc:
        with (
            tc.tile_pool(name="const", bufs=1) as cpool,
            tc.tile_pool(name="weights", bufs=2) as wpool,
            tc.tile_pool(name="x32p", bufs=1) as xpool,
            tc.tile_pool(name="x16p", bufs=2) as x16pool,
            tc.tile_pool(name="acts", bufs=2) as apool,
            tc.tile_pool(name="actT", bufs=2) as atpool,
            tc.tile_pool(name="router", bufs=2) as rpool,
            tc.tile_pool(name="yout", bufs=2) as ypool,
            tc.tile_pool(name="plg", bufs=1, space="PSUM") as plg,
            tc.tile_pool(name="ph", bufs=3, space="PSUM") as pph,
            tc.tile_pool(name="ps", bufs=1, space="PSUM") as pps,
            tc.tile_pool(name="py", bufs=3, space="PSUM") as ppy,
        ):
            id_f32 = cpool.tile([128, 128], F32, tag="id_f32")
            make_identity(nc, id_f32)
            id_bf = cpool.tile([128, 128], BF16, tag="id_bf")
            nc.vector.tensor_copy(id_bf[:], id_f32[:])
            id_r = cpool.tile([128, E], F32R, tag="id_r")
            nc.vector.tensor_copy(id_r[:], id_f32[:, :E])
            negbig = cpool.tile([128, E], F32, tag="negbig")
            nc.vector.memset(negbig, NEG_BIG)
            mgb_sb = cpool.tile([128, NSLOT, E], F32, tag="mgb")
            nc.sync.dma_start(mgb_sb[:], mgb_ap)
            gw_sb = cpool.tile([128, KO, E], F32R, tag="gw")
            nc.sync.dma_start(gw_sb[:], gw_ap)

            xb32 = {}
            xb16 = {}

            def load_batch(bi):
                c0, n = batches[bi]
                w = n * TCH
                t16 = x16pool.tile([128, KO, 256], BF16, tag="x16",
                                   name=f"x16_{bi}")
                nc.sync.dma_start(t16[:, :, :w],
                                  x16_ap[:, :, c0 * TCH:c0 * TCH + w])
                t32 = xpool.tile([128, KO, 256], F32R, tag="x32",
                                 name=f"x32_{bi}")
                nc.sync.dma_start(t32[:, :, :w],
                                  x32_ap[:, :, c0 * TCH:c0 * TCH + w])
                xb32[bi] = t32
                xb16[bi] = t16

            gu_tiles = {}
            dn_tiles = {}

            def load_slot(s):
                gu_t = wpool.tile([128, KO, I2], BF16, tag="gu",
                                  name=f"gu{s}")
                for p in range(3):
                    nc.sync.dma_start(
                        gu_t[:, :, p * HB:(p + 1) * HB],
                        gu_ap[:, :, s * I2 + p * HB:s * I2 + (p + 1) * HB],
                    )
                dn_t = wpool.tile([128, JO, H], BF16, tag="dn",
                                  name=f"dn{s}")
                nc.sync.dma_start(dn_t[:], dn_ap[:, :, s * H:(s + 1) * H])
                gu_tiles[s] = gu_t
                dn_tiles[s] = dn_t

            def emit_router(bi, t32, nch):
                # router logits for the batch (fp32r, ap>=256). One PSUM
                # bank holds both the [E, 256] logits (regions 0:8) and the
                # two per-chunk transposed [128, E] tiles (regions 8:10).
                w = nch * TCH
                lg = plg.tile([128, 10, E], F32, tag="lg", name=f"lg{bi}")
                for k in range(KO):
                    nc.tensor.matmul(lg[:E, :w // E, :], gw_sb[:, k],
                                     t32[:, k, :w],
                                     start=(k == 0), stop=(k == KO - 1))
                lgT = rpool.tile([E, 8, E], F32R, tag="lgT", name=f"lgT{bi}")
                nc.vector.tensor_copy(lgT[:, :w // E, :], lg[:E, :w // E, :])
                for hh in range(nch):
                    nc.tensor.transpose(lg[:, 8 + hh, :].bitcast(F32R),
                                        lgT[:, 4 * hh:4 * (hh + 1), :],
                                        id_r[:E, :E])
                return lg

            def emit_m1_transposes(gc, hh, t16, gu_t):
                # M1 (bf16): h = x16_chunk.T @ gu, b-major; SwiGLU; transpose
                act_sb = apool.tile([128, I], BF16, tag="act",
                                    name=f"act{gc}")
                for b in range(3):
                    h_ps = pph.tile([128, HB], F32, tag="h_ps",
                                    name=f"h{gc}_{b}")
                    for k in range(KO):
                        nc.tensor.matmul(
                            h_ps[:],
                            t16[:, k, hh * TCH:(hh + 1) * TCH],
                            gu_t[:, k, b * HB:(b + 1) * HB],
                            start=(k == 0), stop=(k == KO - 1),
                        )
                    # host interleave: each 512-col block = [256 gate|256 up]
                    silu_sb = apool.tile([128, 256], F32, tag="silu",
                                         name=f"silu{gc}_{b}")
                    nc.scalar.activation(silu_sb[:], h_ps[:, :256],
                                         ACTF.Silu)
                    nc.vector.tensor_tensor(
                        act_sb[:, 256 * b:256 * (b + 1)], silu_sb[:],
                        h_ps[:, 256:], OP.mult,
                    )
                actT = atpool.tile([128, JO, TCH], BF16, tag="actT",
                                   name=f"actT{gc}")
                s_ps = pps.tile([128, JO, TCH], BF16, tag="s_ps",
                                name=f"sps{gc}")
                for j in range(JO):
                    nc.tensor.transpose(s_ps[:, j],
                                        act_sb[:, j * 128:(j + 1) * 128],
                                        id_bf)
                    nc.vector.tensor_copy(actT[:, j], s_ps[:, j])
                return actT

            def emit_rest(gc, hh, s, lgt_ps, actT, dn_t):
                # top-8 chain (DVE; after the actT copies so next chunk's
                # transposes aren't blocked), M2, scale, store
                logits = rpool.tile([128, E], F32, tag="logits",
                                    name=f"logits{gc}")
                nc.vector.tensor_copy(logits[:], lgt_ps[:, 8 + hh, :])
                y_pss = []
                for hb in range(H // HB):
                    y_ps = ppy.tile([128, HB], F32, tag="y_ps",
                                    name=f"y{gc}_{hb}")
                    for j in range(JO):
                        nc.tensor.matmul(
                            y_ps[:], actT[:, j],
                            dn_t[:, j, hb * HB:(hb + 1) * HB],
                            start=(j == 0), stop=(j == JO - 1),
                        )
                    y_pss.append(y_ps)

                cur = rpool.tile([128, E], F32, tag="cur", name=f"cur{gc}")
                nc.vector.tensor_copy(cur[:], logits[:])
                msk = rpool.tile([128, E], U8, tag="msk")
                m1 = rpool.tile([128, 1], F32, tag="m1")
                mk = rpool.tile([128, 1], F32, tag="mk")
                for it in range(TOP_K - 1):
                    tgt = m1 if it == 0 else mk
                    nc.vector.reduce_max(tgt[:], cur[:], axis=AX)
                    nc.vector.tensor_scalar(msk[:], cur[:], tgt[:],
                                            None, OP.is_ge)
                    nc.vector.copy_predicated(cur[:], msk[:], negbig[:])
                m8 = rpool.tile([128, 1], F32, tag="m8")
                nc.vector.reduce_max(m8[:], cur[:], axis=AX)
                nm1 = rpool.tile([128, 1], F32, tag="nm1")
                nc.vector.tensor_scalar(nm1[:], m1[:], -1.0, None, OP.mult)
                mask8 = rpool.tile([128, E], F32, tag="mask8")
                nc.vector.tensor_scalar(mask8[:], logits[:], m8[:],
                                        None, OP.is_ge)
                ew = rpool.tile([128, E], F32, tag="ew")
                nc.scalar.activation(ew[:], logits[:], ACTF.Exp,
                                     bias=nm1[:])
                nc.vector.tensor_tensor(ew[:], ew[:], mask8[:], OP.mult)
                s8 = rpool.tile([128, 1], F32, tag="s8")
                nc.vector.reduce_sum(s8[:], ew[:], axis=AX)
                nc.vector.tensor_tensor(ew[:], ew[:], mgb_sb[:, s], OP.mult)
                num = rpool.tile([128, 1], F32, tag="num")
                nc.vector.reduce_sum(num[:], ew[:], axis=AX)
                rs = rpool.tile([128, 1], F32, tag="rs")
                nc.vector.reciprocal(rs[:], s8[:])
                w_t = rpool.tile([128, 1], F32, tag="w_t")
                nc.vector.tensor_tensor(w_t[:], num[:], rs[:], OP.mult)

                for hb in range(H // HB):
                    y_sb = ypool.tile([128, HB], F32, tag="y_sb",
                                      name=f"ysb{gc}_{hb}")
                    nc.vector.tensor_scalar(y_sb[:], y_pss[hb][:],
                                            w_t[:], None, OP.mult)
                    nc.sync.dma_start(
                        y_d.ap()[gc * TCH:(gc + 1) * TCH,
                                 hb * HB:(hb + 1) * HB], y_sb[:],
                    )

            load_batch(0)
            load_slot(0)

            for bi, (c0, nch) in enumerate(batches):
                t32 = xb32.pop(bi)
                t16 = xb16.pop(bi)

                if bi == 0:
                    # Head: M1 of chunk 0 only needs x16+gu; don't make the
                    # PE wait for the (larger) x32 router stream first.
                    actT0 = emit_m1_transposes(0, 0, t16, gu_tiles[0])
                    lgt_ps = emit_router(bi, t32, nch)
                    if len(batches) > 1:
                        load_batch(1)
                    emit_rest(0, 0, chunk_slot[0], lgt_ps, actT0,
                              dn_tiles[0])
                    start_hh = 1
                else:
                    lgt_ps = emit_router(bi, t32, nch)
                    if bi + 1 < len(batches):
                        load_batch(bi + 1)
                    start_hh = 0

                for hh in range(start_hh, nch):
                    gc = c0 + hh
                    s = chunk_slot[gc]
                    if gc in prefetch_chunk:
                        load_slot(prefetch_chunk[gc])
                    actT = emit_m1_transposes(gc, hh, t16, gu_tiles[s])
                    emit_rest(gc, hh, s, lgt_ps, actT, dn_tiles[s])
    nc.compile()
    _CACHED_NC[sizes] = nc
    return nc


_GATEUP_PERM = np.concatenate(
    [np.r_[256 * b:256 * b + 256, 768 + 256 * b:768 + 256 * b + 256]
     for b in range(3)]
)


def prepare(hidden_states, gate_weight, gate_up_proj, down_proj,
            merge_groups, dominant_experts):
    x = np.asarray(hidden_states, dtype=np.float32).reshape(T, H)
    gw = np.asarray(gate_weight, dtype=np.float32)
    mg = np.asarray(merge_groups).astype(np.int64)
    de = np.asarray(dominant_experts).astype(np.int64)
    gup = np.asarray(gate_up_proj, dtype=np.float32)
    dnp_ = np.asarray(down_proj, dtype=np.float32)

    # Host routing (membership only; device recomputes the weights)
    logits = x @ gw.T
    top8 = np.argpartition(-logits, TOP_K - 1, axis=1)[:, :TOP_K]
    active = np.zeros((T, G), np.bool_)
    active[np.arange(T)[:, None], mg[top8]] = True
    idx_lists = [np.nonzero(active[:, g])[0] for g in range(G)]
    nchunks = [(len(ix) + TCH - 1) // TCH for ix in idx_lists]

    sizes, segcaps = _pack(nchunks)
    c_chunks = sum(sizes)
    C = c_chunks * TCH

    # materialize segments grouped by size class
    segs_by_size = {}
    for g in range(G):
        pos = 0
        for scap in sorted(segcaps[g], reverse=True):
            tk = idx_lists[g][pos:pos + scap * TCH]
            pos += len(tk)
            segs_by_size.setdefault(scap, []).append((g, tk))
        assert pos >= len(idx_lists[g])

    core_slots = [[(None, np.zeros(0, np.int64))] * NSLOT for _ in range(G)]
    for p, v in enumerate(sizes):
        pool = segs_by_size.get(v, [])
        for c in range(G):
            if pool:
                core_slots[c][p] = pool.pop(0)
    for v, pool in segs_by_size.items():
        assert not pool, f"unplaced segments of size {v}"

    xT = np.ascontiguousarray(x.T)
    gwT = np.ascontiguousarray(gw.T)
    guT16 = {}
    dnT16 = {}
    for g in range(G):
        e = int(de[g])
        if e not in guT16:
            guT16[e] = np.ascontiguousarray(
                gup[e].T[:, _GATEUP_PERM]).astype(BF16NP)
            dnT16[e] = np.ascontiguousarray(dnp_[e].T).astype(BF16NP)

    slot_off = [0]
    for v in sizes:
        slot_off.append(slot_off[-1] + v)

    in_maps = []
    scatter = []
    for c in range(G):
        tok = np.zeros(C, np.int64)
        mgb = np.zeros((128, NSLOT * E), np.float32)
        gu_cat = np.zeros((H, NSLOT * I2), BF16NP)
        dn_cat = np.zeros((I, NSLOT * H), BF16NP)
        sc = []
        for p in range(NSLOT):
            g, tk = core_slots[c][p]
            off = slot_off[p] * TCH
            if g is None or len(tk) == 0:
                continue
            tok[off:off + len(tk)] = tk
            mgb[:, p * E:(p + 1) * E] = (mg == g).astype(np.float32)[None, :]
            e = int(de[g])
            gu_cat[:, p * I2:(p + 1) * I2] = guT16[e]
            dn_cat[:, p * H:(p + 1) * H] = dnT16[e]
            sc.append((tk, off))
        x32c = np.ascontiguousarray(xT[:, tok])
        in_maps.append({
            "x32": x32c,
            "x16": x32c.astype(BF16NP),
            "gu": gu_cat,
            "dn": dn_cat,
            "gw": gwT,
            "mgb": mgb,
        })
        scatter.append(sc)
    return sizes, in_maps, scatter


def kernel(hidden_states, gate_weight, gate_up_proj, down_proj,
           merge_groups, dominant_experts):
    sizes, in_maps, scatter = prepare(
        hidden_states, gate_weight, gate_up_proj, down_proj,
        merge_groups, dominant_experts)
    nc = _build(sizes)
    res = run_bass_kernel_spmd(nc, in_maps, core_ids=list(range(G)),
                               trace=False)
    out = np.zeros((T, H), dtype=np.float64)
    for c, r in enumerate(res.results):
        y = r["y"].astype(np.float64)
        for tk, off in scatter[c]:
            out[tk] += y[off:off + len(tk)]
    return out.astype(np.float32).reshape(1, T, H)


# revision 20
# speedup vs baseline: 1.7751x; 1.0188x over previous
"""TRN2 Bass kernel for nn_HCSMoEQwen3MoeSparseMoeBlock (8-core, load-balanced
expert-parallel with capacity slots).

Observation: only ~66% of (token, group) pairs are active (a token
contributes to group g only if one of its top-8 experts merges into g), but
per-group token counts are skewed, so plain expert-parallel (core g = group
g, all tokens) wastes ~34% of the FLOPs. Instead:

- Host computes the routing membership (exact fp32, redundant copy of the
  router purely for data placement) and packs per-group token segments into
  8 cores x 3 uniform weight "slots" of (s0,s1,s2) 128-token chunks
  (DFS packer, ladder of slot shapes; (4,4,3) for the staged data = 1408
  tokens/core vs 2048).
- Device (same program on all 8 cores): for each slot, stream that slot's
  expert weights (bf16, double-buffered); router logits in fp32r on
  256-token batches (ap>=256 keeps the PE at 1 cycle/row; the separate
  fp32 x stream exists only for the router); M1/SwiGLU/transpose/M2 in
  bf16 (1 cycle/row, halved SBUF+DMA); on-device top-8 + group weight;
  w*y stored per chunk.
- Host scatter-adds each slot's rows into the full [T, H] output.

All routing math that affects output VALUES runs on device; the host router
only decides which (token, group) pairs each core computes.
"""
import numpy as np
import ml_dtypes

import concourse.bass as bass
import concourse.mybir as mybir
import concourse.tile as tile
from concourse import bacc
from concourse.bass_utils import run_bass_kernel_spmd
from concourse.masks import make_identity

T = 2048
H = 2048
I2 = 1536
I = 768
E = 32
G = 8
NSLOT = 3
TOP_K = 8
KO = H // 128
JO = I // 128
TCH = 128
HB = 512
NEG_BIG = -1.0e9
BF16NP = ml_dtypes.bfloat16

F32 = mybir.dt.float32
F32R = mybir.dt.float32r
BF16 = mybir.dt.bfloat16
U8 = mybir.dt.uint8
AX = mybir.AxisListType.X
OP = mybir.AluOpType
ACTF = mybir.ActivationFunctionType

_CACHED_NC = {}

# Slot-shape ladder: uniform (s0,s1,s2) chunks per core, 8 slots per
# position. First feasible entry wins; (16,16,16) always fits (any group
# needs at most ceil(2048/128)=16 chunks).
_LADDER = [
    (4, 4, 3), (4, 4, 4), (5, 4, 4), (5, 5, 4), (5, 5, 5), (6, 5, 5),
    (6, 6, 5), (6, 6, 6), (7, 7, 6), (7, 7, 7), (8, 8, 7), (8, 8, 8),
    (10, 10, 10), (12, 12, 12), (14, 14, 14), (16, 16, 16),
]


def _dfs(order, i, avail, nchunks):
    """Assign each group a multiset of slot sizes covering its chunk count.

    avail: dict size -> remaining slot count. Returns {group: [sizes]}.
    """
    if i == len(order):
        return {}
    g = order[i]
    n = nchunks[g]
    if n == 0:
        rest = _dfs(order, i + 1, avail, nchunks)
        if rest is None:
            return None
        rest[g] = []
        return rest
    vs = sorted(avail.keys(), reverse=True)
    combos = []

    def gen(idx, covered, cur):
        if covered >= n:
            combos.append((covered - n, len(cur), list(cur)))
            return
        if idx == len(vs):
            return
        v = vs[idx]
        top = min(avail[v] - cur.count(v), (n - covered + v - 1) // v)
        for k in range(top, -1, -1):
            gen(idx + 1, covered + k * v, cur + [v] * k)

    gen(0, 0, [])
    combos.sort(key=lambda c: (c[0], c[1]))
    for _, _, sizes in combos:
        for v in sizes:
            avail[v] -= 1
        rest = _dfs(order, i + 1, avail, nchunks)
        if rest is not None:
            rest[g] = sizes
            return rest
        for v in sizes:
            avail[v] += 1
    return None


def _pack(nchunks):
    for sizes in _LADDER:
        avail = {}
        for v in sizes:
            avail[v] = avail.get(v, 0) + 8
        order = sorted(range(G), key=lambda g: -nchunks[g])
        res = _dfs(order, 0, dict(avail), nchunks)
        if res is not None:
            return sizes, res
    raise RuntimeError("slot packing failed")


def _build(sizes):
    if sizes in _CACHED_NC:
        return _CACHED_NC[sizes]
    c_chunks = sum(sizes)
    C = c_chunks * TCH
    # 256-token router batches over the whole token range; the first batch
    # is a single chunk so the PE can start after a minimal DMA head
    batches = [(0, 1)]
    c = 1
    while c < c_chunks:
        n = min(2, c_chunks - c)
        batches.append((c, n))
        c += n
    chunk_slot = []
    for s, v in enumerate(sizes):
        chunk_slot += [s] * v
    slot_start = [0]
    for v in sizes[:-1]:
        slot_start.append(slot_start[-1] + v)
    # prefetch slot s one chunk into slot s-1
    prefetch_chunk = {slot_start[s - 1] + 1: s for s in range(1, NSLOT)}

    nc = bacc.Bacc("TRN2", target_bir_lowering=False, debug=False,
                   num_devices=G)

    x32_d = nc.dram_tensor("x32", [H, C], F32R, kind="ExternalInput")
    x16_d = nc.dram_tensor("x16", [H, C], BF16, kind="ExternalInput")
    gu_d = nc.dram_tensor("gu", [H, NSLOT * I2], BF16, kind="ExternalInput")
    dn_d = nc.dram_tensor("dn", [I, NSLOT * H], BF16, kind="ExternalInput")
    gw_d = nc.dram_tensor("gw", [H, E], F32R, kind="ExternalInput")
    mgb_d = nc.dram_tensor("mgb", [128, NSLOT * E], F32, kind="ExternalInput")
    y_d = nc.dram_tensor("y", [C, H], F32, kind="ExternalOutput")

    x32_ap = x32_d.ap().rearrange("(ko p) t -> p ko t", p=128)
    x16_ap = x16_d.ap().rearrange("(ko p) t -> p ko t", p=128)
    gu_ap = gu_d.ap().rearrange("(ko p) o -> p ko o", p=128)
    dn_ap = dn_d.ap().rearrange("(jo p) h -> p jo h", p=128)
    gw_ap = gw_d.ap().rearrange("(ko p) e -> p ko e", p=128)
    mgb_ap = mgb_d.ap().rearrange("p (s e) -> p s e", e=E)

    with tile.TileContext(nc) as tc:
        with (
            tc.tile_pool(name="const", bufs=1) as cpool,
            tc.tile_pool(name="weights", bufs=2) as wpool,
            tc.tile_pool(name="x32p", bufs=1) as xpool,
            tc.tile_pool(name="x16p", bufs=2) as x16pool,
            tc.tile_pool(name="acts", bufs=2) as apool,
            tc.tile_pool(name="actT", bufs=2) as atpool,
            tc.tile_pool(name="router", bufs=2) as rpool,
            tc.tile_pool(name="yout", bufs=2) as ypool,
            tc.tile_pool(name="plg", bufs=1, space="PSUM") as plg,
            tc.tile_pool(name="ph", bufs=2, space="PSUM") as pph,
            tc.tile_pool(name="ps", bufs=2, space="PSUM") as pps,
            tc.tile_pool(name="py", bufs=3, space="PSUM") as ppy,
        ):
            id_f32 = cpool.tile([128, 128], F32, tag="id_f32")
            make_identity(nc, id_f32)
            id_bf = cpool.tile([128, 128], BF16, tag="id_bf")
            nc.vector.tensor_copy(id_bf[:], id_f32[:])
            id_r = cpool.tile([128, E], F32R, tag="id_r")
            nc.vector.tensor_copy(id_r[:], id_f32[:, :E])
            negbig = cpool.tile([128, E], F32, tag="negbig")
            nc.vector.memset(negbig, NEG_BIG)
            mgb_sb = cpool.tile([128, NSLOT, E], F32, tag="mgb")
            nc.sync.dma_start(mgb_sb[:], mgb_ap)
            gw_sb = cpool.tile([128, KO, E], F32R, tag="gw")
            nc.sync.dma_start(gw_sb[:], gw_ap)

            xb32 = {}
            xb16 = {}

            def load_batch(bi):
                c0, n = batches[bi]
                w = n * TCH
                t16 = x16pool.tile([128, KO, 256], BF16, tag="x16",
                                   name=f"x16_{bi}")
                nc.sync.dma_start(t16[:, :, :w],
                                  x16_ap[:, :, c0 * TCH:c0 * TCH + w])
                t32 = xpool.tile([128, KO, 256], F32R, tag="x32",
                                 name=f"x32_{bi}")
                nc.sync.dma_start(t32[:, :, :w],
                                  x32_ap[:, :, c0 * TCH:c0 * TCH + w])
                xb32[bi] = t32
                xb16[bi] = t16

            gu_tiles = {}
            dn_tiles = {}

            def load_slot(s):
                gu_t = wpool.tile([128, KO, I2], BF16, tag="gu",
                                  name=f"gu{s}")
                for p in range(3):
                    nc.sync.dma_start(
                        gu_t[:, :, p * HB:(p + 1) * HB],
                        gu_ap[:, :, s * I2 + p * HB:s * I2 + (p + 1) * HB],
                    )
                dn_t = wpool.tile([128, JO, H], BF16, tag="dn",
                                  name=f"dn{s}")
                nc.sync.dma_start(dn_t[:], dn_ap[:, :, s * H:(s + 1) * H])
                gu_tiles[s] = gu_t
                dn_tiles[s] = dn_t

            def emit_router(bi, t32, nch):
                # router logits for the batch (fp32r, ap>=256). One PSUM
                # bank holds both the [E, 256] logits (regions 0:8) and the
                # two per-chunk transposed [128, E] tiles (regions 8:10).
                w = nch * TCH
                lg = plg.tile([128, 10, E], F32, tag="lg", name=f"lg{bi}")
                for k in range(KO):
                    nc.tensor.matmul(lg[:E, :w // E, :], gw_sb[:, k],
                                     t32[:, k, :w],
                                     start=(k == 0), stop=(k == KO - 1))
                lgT = rpool.tile([E, 8, E], F32R, tag="lgT", name=f"lgT{bi}")
                nc.vector.tensor_copy(lgT[:, :w // E, :], lg[:E, :w // E, :])
                for hh in range(nch):
                    nc.tensor.transpose(lg[:, 8 + hh, :].bitcast(F32R),
                                        lgT[:, 4 * hh:4 * (hh + 1), :],
                                        id_r[:E, :E])
                return lg

            def emit_m1_transposes(gc, hh, t16, gu_t):
                # M1 (bf16): h = x16_chunk.T @ gu, b-major; SwiGLU; transpose
                act_sb = apool.tile([128, I], BF16, tag="act",
                                    name=f"act{gc}")
                for b in range(3):
                    h_ps = pph.tile([128, HB], F32, tag="h_ps",
                                    name=f"h{gc}_{b}")
                    for k in range(KO):
                        nc.tensor.matmul(
                            h_ps[:],
                            t16[:, k, hh * TCH:(hh + 1) * TCH],
                            gu_t[:, k, b * HB:(b + 1) * HB],
                            start=(k == 0), stop=(k == KO - 1),
                        )
                    # host interleave: each 512-col block = [256 gate|256 up]
                    silu_sb = apool.tile([128, 256], F32, tag="silu",
                                         name=f"silu{gc}_{b}")
                    nc.scalar.activation(silu_sb[:], h_ps[:, :256],
                                         ACTF.Silu)
                    nc.vector.tensor_tensor(
                        act_sb[:, 256 * b:256 * (b + 1)], silu_sb[:],
                        h_ps[:, 256:], OP.mult,
                    )
                actT = atpool.tile([128, JO, TCH], BF16, tag="actT",
                                   name=f"actT{gc}")
                s_ps = pps.tile([128, JO, TCH], BF16, tag="s_ps",
                                name=f"sps{gc}")
                for j in range(JO):
                    nc.tensor.transpose(s_ps[:, j],
                                        act_sb[:, j * 128:(j + 1) * 128],
                                        id_bf)
                    nc.vector.tensor_copy(actT[:, j], s_ps[:, j])
                return actT

            def emit_chain(gc, hh, s, lgt_ps):
                # top-8 + group-weight chain (DVE + one scalar Exp). Issued
                # before M1 so w_t is ready well before the y scales.
                logits = rpool.tile([128, E], F32, tag="logits",
                                    name=f"logits{gc}")
                nc.vector.tensor_copy(logits[:], lgt_ps[:, 8 + hh, :])
                cur = rpool.tile([128, E], F32, tag="cur", name=f"cur{gc}")
                nc.vector.tensor_copy(cur[:], logits[:])
                msk = rpool.tile([128, E], U8, tag="msk")
                m1 = rpool.tile([128, 1], F32, tag="m1")
                mk = rpool.tile([128, 1], F32, tag="mk")
                for it in range(TOP_K - 1):
                    tgt = m1 if it == 0 else mk
                    nc.vector.reduce_max(tgt[:], cur[:], axis=AX)
                    nc.vector.tensor_scalar(msk[:], cur[:], tgt[:],
                                            None, OP.is_ge)
                    nc.vector.copy_predicated(cur[:], msk[:], negbig[:])
                m8 = rpool.tile([128, 1], F32, tag="m8")
                nc.vector.reduce_max(m8[:], cur[:], axis=AX)
                nm1 = rpool.tile([128, 1], F32, tag="nm1")
                nc.vector.tensor_scalar(nm1[:], m1[:], -1.0, None, OP.mult)
                mask8 = rpool.tile([128, E], F32, tag="mask8")
                nc.vector.tensor_scalar(mask8[:], logits[:], m8[:],
                                        None, OP.is_ge)
                ew = rpool.tile([128, E], F32, tag="ew")
                nc.scalar.activation(ew[:], logits[:], ACTF.Exp,
                                     bias=nm1[:])
                nc.vector.tensor_tensor(ew[:], ew[:], mask8[:], OP.mult)
                s8 = rpool.tile([128, 1], F32, tag="s8")
                nc.vector.reduce_sum(s8[:], ew[:], axis=AX)
                nc.vector.tensor_tensor(ew[:], ew[:], mgb_sb[:, s], OP.mult)
                num = rpool.tile([128, 1], F32, tag="num")
                nc.vector.reduce_sum(num[:], ew[:], axis=AX)
                rs = rpool.tile([128, 1], F32, tag="rs")
                nc.vector.reciprocal(rs[:], s8[:])
                w_t = rpool.tile([128, 1], F32, tag="w_t",
                                 name=f"w_t{gc}")
                nc.vector.tensor_tensor(w_t[:], num[:], rs[:], OP.mult)
                return w_t

            def emit_m2_store(gc, w_t, actT, dn_t):
                for hb in range(H // HB):
                    y_ps = ppy.tile([128, HB], F32, tag="y_ps",
                                    name=f"y{gc}_{hb}")
                    for j in range(JO):
                        nc.tensor.matmul(
                            y_ps[:], actT[:, j],
                            dn_t[:, j, hb * HB:(hb + 1) * HB],
                            start=(j == 0), stop=(j == JO - 1),
                        )
                    y_sb = ypool.tile([128, HB], F32, tag="y_sb",
                                      name=f"ysb{gc}_{hb}")
                    nc.vector.tensor_scalar(y_sb[:], y_ps[:],
                                            w_t[:], None, OP.mult)
                    nc.sync.dma_start(
                        y_d.ap()[gc * TCH:(gc + 1) * TCH,
                                 hb * HB:(hb + 1) * HB], y_sb[:],
                    )

            load_batch(0)
            load_slot(0)

            for bi, (c0, nch) in enumerate(batches):
                t32 = xb32.pop(bi)
                t16 = xb16.pop(bi)

                if bi == 0:
                    # Head: M1 of chunk 0 only needs x16+gu; don't make the
                    # PE wait for the (larger) x32 router stream first.
                    actT0 = emit_m1_transposes(0, 0, t16, gu_tiles[0])
                    lgt_ps = emit_router(bi, t32, nch)
                    if len(batches) > 1:
                        load_batch(1)
                    w_t0 = emit_chain(0, 0, chunk_slot[0], lgt_ps)
                    emit_m2_store(0, w_t0, actT0, dn_tiles[0])
                    start_hh = 1
                else:
                    lgt_ps = emit_router(bi, t32, nch)
                    if bi + 1 < len(batches):
                        load_batch(bi + 1)
                    start_hh = 0

                for hh in range(start_hh, nch):
                    gc = c0 + hh
                    s = chunk_slot[gc]
                    if gc in prefetch_chunk:
                        load_slot(prefetch_chunk[gc])
                    w_t = emit_chain(gc, hh, s, lgt_ps)
                    actT = emit_m1_transposes(gc, hh, t16, gu_tiles[s])
                    emit_m2_store(gc, w_t, actT, dn_tiles[s])
    nc.compile()
    _CACHED_NC[sizes] = nc
    return nc


_GATEUP_PERM = np.concatenate(
    [np.r_[256 * b:256 * b + 256, 768 + 256 * b:768 + 256 * b + 256]
     for b in range(3)]
)


def prepare(hidden_states, gate_weight, gate_up_proj, down_proj,
            merge_groups, dominant_experts):
    x = np.asarray(hidden_states, dtype=np.float32).reshape(T, H)
    gw = np.asarray(gate_weight, dtype=np.float32)
    mg = np.asarray(merge_groups).astype(np.int64)
    de = np.asarray(dominant_experts).astype(np.int64)
    gup = np.asarray(gate_up_proj, dtype=np.float32)
    dnp_ = np.asarray(down_proj, dtype=np.float32)

    # Host routing (membership only; device recomputes the weights)
    logits = x @ gw.T
    top8 = np.argpartition(-logits, TOP_K - 1, axis=1)[:, :TOP_K]
    active = np.zeros((T, G), np.bool_)
    active[np.arange(T)[:, None], mg[top8]] = True
    idx_lists = [np.nonzero(active[:, g])[0] for g in range(G)]
    nchunks = [(len(ix) + TCH - 1) // TCH for ix in idx_lists]

    sizes, segcaps = _pack(nchunks)
    c_chunks = sum(sizes)
    C = c_chunks * TCH

    # materialize segments grouped by size class
    segs_by_size = {}
    for g in range(G):
        pos = 0
        for scap in sorted(segcaps[g], reverse=True):
            tk = idx_lists[g][pos:pos + scap * TCH]
            pos += len(tk)
            segs_by_size.setdefault(scap, []).append((g, tk))
        assert pos >= len(idx_lists[g])

    core_slots = [[(None, np.zeros(0, np.int64))] * NSLOT for _ in range(G)]
    for p, v in enumerate(sizes):
        pool = segs_by_size.get(v, [])
        for c in range(G):
            if pool:
                core_slots[c][p] = pool.pop(0)
    for v, pool in segs_by_size.items():
        assert not pool, f"unplaced segments of size {v}"

    xT = np.ascontiguousarray(x.T)
    gwT = np.ascontiguousarray(gw.T)
    guT16 = {}
    dnT16 = {}
    for g in range(G):
        e = int(de[g])
        if e not in guT16:
            guT16[e] = np.ascontiguousarray(
                gup[e].T[:, _GATEUP_PERM]).astype(BF16NP)
            dnT16[e] = np.ascontiguousarray(dnp_[e].T).astype(BF16NP)

    slot_off = [0]
    for v in sizes:
        slot_off.append(slot_off[-1] + v)

    in_maps = []
    scatter = []
    for c in range(G):
        tok = np.zeros(C, np.int64)
        mgb = np.zeros((128, NSLOT * E), np.float32)
        gu_cat = np.zeros((H, NSLOT * I2), BF16NP)
        dn_cat = np.zeros((I, NSLOT * H), BF16NP)
        sc = []
        for p in range(NSLOT):
            g, tk = core_slots[c][p]
            off = slot_off[p] * TCH
            if g is None or len(tk) == 0:
                continue
            tok[off:off + len(tk)] = tk
            mgb[:, p * E:(p + 1) * E] = (mg == g).astype(np.float32)[None, :]
            e = int(de[g])
            gu_cat[:, p * I2:(p + 1) * I2] = guT16[e]
            dn_cat[:, p * H:(p + 1) * H] = dnT16[e]
            sc.append((tk, off))
        x32c = np.ascontiguousarray(xT[:, tok])
        in_maps.append({
            "x32": x32c,
            "x16": x32c.astype(BF16NP),
            "gu": gu_cat,
            "dn": dn_cat,
            "gw": gwT,
            "mgb": mgb,
        })
        scatter.append(sc)
    return sizes, in_maps, scatter


def kernel(hidden_states, gate_weight, gate_up_proj, down_proj,
           merge_groups, dominant_experts):
    sizes, in_maps, scatter = prepare(
        hidden_states, gate_weight, gate_up_proj, down_proj,
        merge_groups, dominant_experts)
    nc = _build(sizes)
    res = run_bass_kernel_spmd(nc, in_maps, core_ids=list(range(G)),
                               trace=False)
    out = np.zeros((T, H), dtype=np.float64)
    for c, r in enumerate(res.results):
        y = r["y"].astype(np.float64)
        for tk, off in scatter[c]:
            out[tk] += y[off:off + len(tk)]
    return out.astype(np.float32).reshape(1, T, H)
